# revision 26
# baseline (speedup 1.0000x reference)
"""Trainium2 Bass kernel for nn_DQNConv (conv stack -> linear -> legal-move
masked softmax), data-parallel over 8 NeuronCores.

Self-contained: takes FULL inputs as numpy arrays, shards batch across the 8
cores, runs one SPMD Bass program, returns the FULL [16384, 4096] float32
output.

Algorithm per core (2048 rows), per 512-row chunk:
  - conv1 (7x7x1 -> 5x5x32) as 10 matmuls producing overlapping pixel-window
    tiles hA[y']=pixels(y',0..3)x32ch and hB[y']=pixels(y',1..4)x32ch so that
    conv2's 3x3 contraction windows are 128-partition aligned.
  - conv2 (-> 3x3x64) as 18 matmuls: out tiles {(oy,0),(oy,1)}x64 (128 rows)
    and {(oy,2)}x64 (64 rows), each accumulating 3 ky-blocks.
  - conv3 (-> 1x1x64) as 6 matmuls contracting the 6 h2 tiles.
  - ReLU+bias evacuations are split between ScalarE and VectorE to balance
    those two engines (both are near the DMA/GPSIMD roofline).
  - logits tile [128 rows, 4096] on TensorE (f32r), then the legal-move mask
    (built by GPSIMD local_scatter as bf16 0/1) is folded into PSUM by an
    extra matmul against a 30*I bf16 identity: psum = logits + 30*mask.
  - e = exp(psum - 30) on ScalarE: legal entries get exp(logit) exactly,
    illegal entries underflow to ~e^-30 (abs err <= 1e-13 vs the reference's
    exact 0, far inside the 2e-2 gate).  This removes the mask-multiply pass.
  - VectorE: row-sum Z (identity tensor_scalar with accum_out at 4x bf16
    rate), reciprocal, o = e * (1/Z) in f32, straight HWDGE DMA to HBM.
"""

import sys
import os

for _p in ("/opt/trn_rl_repo", "/root/.axon_site/_ro/trn_rl_repo"):
    if os.path.isdir(_p) and _p not in sys.path:
        sys.path.append(_p)

import numpy as np

import concourse.bass as bass
import concourse.bacc as bacc
import concourse.mybir as mybir
import concourse.tile as tile
from concourse.bass_utils import run_bass_kernel_spmd

B, HW, OUT, K = 16384, 7, 4096, 64
NCORES = 8
BC = B // NCORES           # 2048 rows per core
NRT = BC // 128            # 16 row-tiles per core
NCHUNK = 4                 # conv batch chunks per core
CW = BC // NCHUNK          # 512 columns per conv chunk (f32r full-rate >=256)
F0 = 49
MB = 30.0                  # mask logit boost; exp(-MB) ~ 1e-13 fakes the zeros
SCATTER_CHUNKS = [(0, 2046), (2046, 2046), (4092, 4)]

dt = mybir.dt
AT = mybir.AluOpType
ACTF = mybir.ActivationFunctionType
F32R = dt.float32r
BF16 = dt.bfloat16

# variant switches for hardware bisection
V_INJECT = os.environ.get("KERNEL_INJECT", "1") == "1"   # mask via PE inject
V_DVE_EVAC = os.environ.get("KERNEL_DVE_EVAC", "0") == "1"  # some evacs on DVE
V_FASTRECIP = os.environ.get("KERNEL_FASTRECIP", "1") == "1"
V_SCATTER = os.environ.get("KERNEL_SCATTER", "1") == "1"  # 0: memset (timing only)
V_ACT_ACCUM = os.environ.get("KERNEL_ACT_ACCUM", "0") == "1"  # Z via exp accum_out
V_HALFOUT = os.environ.get("KERNEL_HALFOUT", "0") == "1"  # half-width out DMA (timing probe)
V_HOSTMASK = os.environ.get("KERNEL_HOSTMASK", "0") == "1"  # host-built mask via DMA


def _build(reps=1, fori=0):
    nc = bacc.Bacc("TRN2", target_bir_lowering=False, debug=False)

    xT = nc.dram_tensor("xT", [F0, BC], F32R, kind="ExternalInput")
    m1 = nc.dram_tensor("m1", [F0, 10 * 128], F32R, kind="ExternalInput")
    w2a = nc.dram_tensor("w2a", [128, 3 * 128], F32R, kind="ExternalInput")
    w2b = nc.dram_tensor("w2b", [128, 3 * 64], F32R, kind="ExternalInput")
    m3 = nc.dram_tensor("m3", [128, 6 * 64], F32R, kind="ExternalInput")
    wlT = nc.dram_tensor("wlT", [65, OUT], F32R, kind="ExternalInput")
    bad = nc.dram_tensor("ball", [128, 13], dt.float32, kind="ExternalInput")
    eyed = nc.dram_tensor("eye", [128, 128], BF16, kind="ExternalInput")
    if V_HOSTMASK:
        mskd = nc.dram_tensor("mskd", [128, NRT * OUT], BF16, kind="ExternalInput")
    else:
        pmi = nc.dram_tensor("pmi", [128, NRT * 3 * K], dt.int16, kind="ExternalInput")
    outd = nc.dram_tensor("out", [BC, OUT], dt.float32, kind="ExternalOutput")

    with tile.TileContext(nc) as tc:
        with (
            tc.tile_pool(name="w", bufs=1) as wp,
            tc.tile_pool(name="h", bufs=2) as hp,
            tc.tile_pool(name="m", bufs=3) as mp,
            tc.tile_pool(name="b", bufs=2) as bp,
            tc.tile_pool(name="pc", bufs=4, space="PSUM") as pp,
            tc.tile_pool(name="pl", bufs=2, space="PSUM") as pl,
        ):
            # ---- static loads (outside the timing loop) -----------------------
            m1_sb = wp.tile([F0, 10 * 128], F32R, tag="m1")
            nc.sync.dma_start(out=m1_sb[:], in_=m1.ap())
            ball_sb = wp.tile([128, 13], dt.float32, tag="ball")
            nc.sync.dma_start(out=ball_sb[:], in_=bad.ap())
            xT_sb = wp.tile([F0, BC], F32R, tag="xT")
            nc.sync.dma_start(out=xT_sb[:], in_=xT.ap())
            # per-partition-major pmi: partition p holds the 16 row-tiles'
            # index triplets for row  rt*128+p  at [p, rt, :]
            if not V_HOSTMASK:
                ix_sb = wp.tile([128, NRT, 3 * K], dt.int16, tag="ix")
                nc.sync.dma_start(out=ix_sb[:], in_=pmi.ap())
            w2a_sb = wp.tile([128, 3 * 128], F32R, tag="w2a")
            nc.sync.dma_start(out=w2a_sb[:], in_=w2a.ap())
            w2b_sb = wp.tile([128, 3 * 64], F32R, tag="w2b")
            nc.sync.dma_start(out=w2b_sb[:], in_=w2b.ap())
            m3_sb = wp.tile([128, 6 * 64], F32R, tag="m3")
            nc.sync.dma_start(out=m3_sb[:], in_=m3.ap())
            wl_sb = wp.tile([65, OUT], F32R, tag="wl")
            nc.sync.dma_start(out=wl_sb[:], in_=wlT.ap())
            eye_sb = wp.tile([128, 128], BF16, tag="eye")
            nc.sync.dma_start(out=eye_sb[:], in_=eyed.ap())
            ones_sb = wp.tile([128, K], BF16, tag="ones")
            nc.vector.memset(ones_sb[:], 1.0)
            negb_sb = wp.tile([128, 1], dt.float32, tag="negb")
            nc.vector.memset(negb_sb[:], -MB)
            # feat double-buffer with the homogeneous bias row pre-set
            feat_bufs = []
            for i in range(2):
                f = wp.tile([65, CW], F32R, tag=f"feat{i}")
                nc.vector.memset(f[64:65, :].bitcast(dt.float32), 1.0)
                feat_bufs.append(f)

            b_l1 = [ball_sb[:, t:t + 1] for t in range(10)]
            b_l2a = ball_sb[:, 10:11]
            b_l2b = ball_sb[:64, 11:12]
            b_l3 = ball_sb[:64, 12:13]

            # evacuate PSUM -> SBUF with ReLU(x + bias) on ACT or DVE
            def evac(dst, src, bias, on_act):
                if on_act or not V_DVE_EVAC:
                    nc.scalar.activation(dst, src, ACTF.Relu, bias=bias)
                else:
                    nc.vector.tensor_scalar(
                        out=dst, in0=src, scalar1=bias, scalar2=0.0,
                        op0=AT.add, op1=AT.max)

            # ---- chunk bodies -------------------------------------------------
            def conv(c):
                """conv1->conv2->conv3 for batch chunk c into feat_bufs[c%2]."""
                cs = slice(c * CW, (c + 1) * CW)
                # L1: 10 window tiles (hA[y'], hB[y'] for y'=0..4)
                h1 = []
                for t in range(10):
                    ps = pp.tile([128, CW], dt.float32, tag="ps")
                    nc.tensor.matmul(
                        ps[:], m1_sb[:, t * 128:(t + 1) * 128], xT_sb[:, cs],
                        start=True, stop=True)
                    h = hp.tile([128, CW], F32R, tag=f"h1_{t}")
                    evac(h[:], ps[:], b_l1[t], on_act=(t % 2 == 0))
                    h1.append(h)
                hA = [h1[2 * y] for y in range(5)]
                hB = [h1[2 * y + 1] for y in range(5)]

                # L2: per output row oy: a 128-row pair tile + a 64-row tile
                h2a, h2b = [], []
                for oy in range(3):
                    psa = pp.tile([128, CW], dt.float32, tag="ps")
                    for ky in range(3):
                        nc.tensor.matmul(
                            psa[:], w2a_sb[:, ky * 128:(ky + 1) * 128],
                            hA[oy + ky][:],
                            start=(ky == 0), stop=(ky == 2))
                    ha = hp.tile([128, CW], F32R, tag=f"h2a_{oy}")
                    evac(ha[:], psa[:], b_l2a, on_act=(oy % 2 == 0))
                    h2a.append(ha)
                    psb = pp.tile([64, CW], dt.float32, tag="ps")
                    for ky in range(3):
                        nc.tensor.matmul(
                            psb[:], w2b_sb[:, ky * 64:(ky + 1) * 64],
                            hB[oy + ky][:],
                            start=(ky == 0), stop=(ky == 2))
                    hb = hp.tile([64, CW], F32R, tag=f"h2b_{oy}")
                    evac(hb[:], psb[:], b_l2b, on_act=(oy % 2 == 1))
                    h2b.append(hb)

                # L3: contract the 6 h2 tiles -> feat chunk [65, CW]
                ps3 = pp.tile([64, CW], dt.float32, tag="ps")
                for oy in range(3):
                    nc.tensor.matmul(
                        ps3[:], m3_sb[:, (2 * oy) * 64:(2 * oy + 1) * 64],
                        h2a[oy][:], start=(oy == 0), stop=False)
                    nc.tensor.matmul(
                        ps3[:], m3_sb[:64, (2 * oy + 1) * 64:(2 * oy + 2) * 64],
                        h2b[oy][:], start=False, stop=(oy == 2))
                evac(feat_bufs[c % 2][:64, :], ps3[:], b_l3, on_act=False)

            def phaseB(c):
                """masked softmax for the 4 row-tiles of chunk c."""
                feat = feat_bufs[c % 2]
                for r in range(CW // 128):
                    rt = c * (CW // 128) + r
                    lhsT = feat[:, r * 128:(r + 1) * 128]

                    msk = mp.tile([128, OUT], BF16, tag="msk")
                    if V_HOSTMASK:
                        nc.sync.dma_start(
                            out=msk[:],
                            in_=mskd.ap()[:, rt * OUT:(rt + 1) * OUT])
                    elif V_SCATTER:
                        for ci, (base, size) in enumerate(SCATTER_CHUNKS):
                            nc.gpsimd.local_scatter(
                                out_ap=msk[:, base:base + size],
                                data_ap=ones_sb[:],
                                idxs_ap=ix_sb[:, rt, ci * K:(ci + 1) * K],
                                channels=128, num_elems=size, num_idxs=K)
                    else:
                        nc.gpsimd.memset(msk[:], 1.0)

                    # psum = logits + MB*mask; e = exp(psum - MB)
                    e = bp.tile([128, OUT], BF16, tag="e")
                    z4 = bp.tile([128, 4], dt.float32, tag="z4")
                    for q in range(4):
                        psl = pl.tile([128, OUT // 4], dt.float32, tag="pl")
                        for nb in range(2):
                            ns = slice(q * 1024 + nb * 512,
                                       q * 1024 + (nb + 1) * 512)
                            nc.tensor.matmul(
                                psl[:, nb * 512:(nb + 1) * 512],
                                lhsT, wl_sb[:, ns],
                                start=True, stop=not V_INJECT)
                            if V_INJECT:
                                nc.tensor.matmul(
                                    psl[:, nb * 512:(nb + 1) * 512],
                                    eye_sb[:], msk[:, ns],
                                    start=False, stop=True)
                        nc.scalar.activation(
                            e[:, q * 1024:(q + 1) * 1024], psl[:], ACTF.Exp,
                            bias=negb_sb[:] if V_INJECT else 0.0,
                            accum_out=z4[:, q:q + 1] if V_ACT_ACCUM else None)

                    if not V_INJECT:
                        nc.vector.tensor_mul(e[:], e[:], msk[:])
                    # Z per row, then o = e * (1/Z)
                    z = bp.tile([128, 1], dt.float32, tag="z")
                    if V_ACT_ACCUM:
                        nc.vector.tensor_scalar(
                            out=z4[:], in0=z4[:], scalar1=1.0, scalar2=0.0,
                            op0=AT.mult, op1=AT.add, accum_out=z[:])
                    else:
                        nc.vector.tensor_scalar(
                            out=e[:], in0=e[:], scalar1=1.0, scalar2=0.0,
                            op0=AT.mult, op1=AT.add, accum_out=z[:])
                    rz = bp.tile([128, 1], dt.float32, tag="rz")
                    if V_FASTRECIP:
                        nc.vector.reciprocal_approx_fast(out=rz[:], in_=z[:])
                    else:
                        nc.vector.reciprocal(rz[:], z[:])
                    o = bp.tile([128, OUT], dt.float32, tag="o")
                    nc.vector.tensor_scalar(
                        out=o[:], in0=e[:], scalar1=rz[:], scalar2=None,
                        op0=AT.mult)
                    if V_HALFOUT:
                        nc.sync.dma_start(
                            out=outd.ap()[rt * 128:(rt + 1) * 128, :OUT // 2],
                            in_=o[:, :OUT // 2])
                    else:
                        nc.sync.dma_start(
                            out=outd.ap()[rt * 128:(rt + 1) * 128, :], in_=o[:])

            # ---- pipelined emission: conv(c+1) ahead of phaseB(c) -------------
            # Rotated so the body also pre-computes conv(0) for the NEXT
            # iteration: every chunk boundary (incl. the loop edge) overlaps.
            import contextlib
            rot = os.environ.get("KERNEL_ROT", "1") == "1"
            _loop = tc.For_i(0, fori, 1) if fori > 0 else contextlib.nullcontext()
            if rot:
                conv(0)
            with _loop:
             for _rep in range(reps):
                if rot:
                    conv(1)
                    phaseB(0)
                    conv(2)
                    phaseB(1)
                    conv(3)
                    phaseB(2)
                    conv(0)
                    phaseB(3)
                else:
                    conv(0)
                    conv(1)
                    phaseB(0)
                    conv(2)
                    phaseB(1)
                    conv(3)
                    phaseB(2)
                    phaseB(3)

    nc.compile()
    return nc


_CACHE = {}


def _get_nc(reps=1, fori=0):
    key = ("nc", reps, fori)
    if key not in _CACHE:
        _CACHE[key] = _build(reps, fori)
    return _CACHE[key]


def _pack_weights(W1, b1, W2, b2, W3, b3):
    """Pack conv weights into the window-tile matmul operands."""
    # L1: M1 tiles [49, 10*128]: tile 2*y'=hA[y'] (out pixels (y',0..3)),
    # tile 2*y'+1=hB[y'] (out pixels (y',1..4)); row = iy*7+ix, col = xi*32+c
    m1 = np.zeros((F0, 10 * 128), np.float32)
    for yp in range(5):
        for half, x0 in ((0, 0), (1, 1)):
            t = 2 * yp + half
            for xi in range(4):
                ox = x0 + xi
                for ky in range(3):
                    for kx in range(3):
                        m1[(yp + ky) * 7 + (ox + kx),
                           t * 128 + xi * 32 + np.arange(32)] = W1[:, 0, ky, kx]
    # L2a: [128, 3*128]: row = xi*32+ic (input x=xi in hA), col = ky*128+ox*64+oc
    w2a = np.zeros((128, 3 * 128), np.float32)
    w2b = np.zeros((128, 3 * 64), np.float32)
    for ky in range(3):
        for xi in range(4):
            for ox in range(2):
                kx = xi - ox
                if 0 <= kx <= 2:
                    # W2[oc, ic, ky, kx] -> rows xi*32+ic, cols ky*128+ox*64+oc
                    w2a[xi * 32 + np.arange(32)[:, None],
                        ky * 128 + ox * 64 + np.arange(64)[None, :]] = \
                        W2[:, :, ky, kx].T
            kx = (xi + 1) - 2      # hB rows are input x = xi+1; out ox = 2
            if 0 <= kx <= 2:
                w2b[xi * 32 + np.arange(32)[:, None],
                    ky * 64 + np.arange(64)[None, :]] = W2[:, :, ky, kx].T
    # L3: [128, 6*64]: col block 2*oy = from h2a[oy] (rows ox*64+oc2),
    # block 2*oy+1 = from h2b[oy] (rows oc2)
    m3 = np.zeros((128, 6 * 64), np.float32)
    for oy in range(3):
        for ox in range(2):
            m3[ox * 64 + np.arange(64)[:, None],
               (2 * oy) * 64 + np.arange(64)[None, :]] = W3[:, :, oy, ox].T
        m3[np.arange(64)[:, None],
           (2 * oy + 1) * 64 + np.arange(64)[None, :]] = W3[:, :, oy, 2].T
    # biases: 10 L1 columns (b1 per channel at xi*32+c), L2a, L2b, L3
    ball = np.zeros((128, 13), np.float32)
    for t in range(10):
        ball[:, t] = np.tile(b1, 4)
    ball[:, 10] = np.tile(b2, 2)
    ball[:64, 11] = b2
    ball[:64, 12] = b3
    return m1, w2a, w2b, m3, ball


def kernel(**inputs):
    x = np.ascontiguousarray(np.asarray(inputs["x"], dtype=np.float32)).reshape(B, F0)
    pm = np.asarray(inputs["possible_moves"]).astype(np.int32, copy=False)
    W1 = np.asarray(inputs["W1"], dtype=np.float32)
    b1 = np.asarray(inputs["b1"], dtype=np.float32)
    W2 = np.asarray(inputs["W2"], dtype=np.float32)
    b2 = np.asarray(inputs["b2"], dtype=np.float32)
    W3 = np.asarray(inputs["W3"], dtype=np.float32)
    b3 = np.asarray(inputs["b3"], dtype=np.float32)
    Wl = np.asarray(inputs["Wl"], dtype=np.float32)
    bl = np.asarray(inputs["bl"], dtype=np.float32)

    import ml_dtypes
    m1, w2a, w2b, m3, ball = _pack_weights(W1, b1, W2, b2, W3, b3)
    WlT = np.concatenate([Wl.T.astype(np.float32), bl[None, :]], axis=0)
    eye = (MB * np.eye(128, dtype=np.float32)).astype(ml_dtypes.bfloat16)

    if V_HOSTMASK:
        # host-built 0/1 mask, partition-major [128, rowtile, 4096] bf16
        mall = np.zeros((B, OUT), ml_dtypes.bfloat16)
        mall[np.arange(B)[:, None], pm] = 1.0
        mall = mall.reshape(B // 128, 128, OUT).transpose(1, 0, 2)
    else:
        # per-row scatter indices, chunked to local_scatter's num_elems limit,
        # then repacked partition-major: [p, rt, 3K] holds row rt*128+p
        pmi = np.empty((B, 3, K), np.int16)
        for ci, (base, size) in enumerate(SCATTER_CHUNKS):
            inr = (pm >= base) & (pm < base + size)
            pmi[:, ci, :] = np.where(inr, pm - base, -1).astype(np.int16)
        pmi = pmi.reshape(B // 128, 128, 3 * K).transpose(1, 0, 2)

    xTall = np.ascontiguousarray(x.T)   # [49, B]

    nc = _get_nc()
    in_maps = []
    for c in range(NCORES):
        sl = slice(c * BC, (c + 1) * BC)
        im = {
            "xT": np.ascontiguousarray(xTall[:, sl]),
            "m1": m1, "w2a": w2a, "w2b": w2b, "m3": m3, "wlT": WlT,
            "ball": ball, "eye": eye,
        }
        if V_HOSTMASK:
            im["mskd"] = np.ascontiguousarray(
                mall[:, c * NRT:(c + 1) * NRT, :].reshape(128, NRT * OUT))
        else:
            im["pmi"] = np.ascontiguousarray(
                pmi[:, c * NRT:(c + 1) * NRT, :].reshape(128, NRT * 3 * K))
        in_maps.append(im)

    trace = bool(int(os.environ.get("KERNEL_TRACE", "0")))
    res = run_bass_kernel_spmd(nc, in_maps, list(range(NCORES)), trace=trace)
    _CACHE["last_results"] = res
    out = np.concatenate([res.results[i]["out"] for i in range(NCORES)], axis=0)
    return out
